# revision 4
# baseline (speedup 1.0000x reference)
"""Trainium2 Bass kernel for CustomConv2d:
  x [16, 32, 512, 512] f32, weight [32, 32, 3, 3] f32, bias [32] f32
  -> out [16, 32, 510, 510] f32   (stride 1, VALID padding, + bias)

Data-parallel over batch: 2 images per core across 8 NeuronCores.

v4 design (bf16 I/O + 16-tile PE with width-split, 100%-useful matmuls):
 - Host converts x/weight to bf16 and relayouts x into the exact SBUF strip
   layout, so every input DMA is one fully-contiguous ~2.1MB transfer per
   strip tile. Output is dumped as packed [128, 512] bf16 drain tiles and
   unshuffled + upcast to f32 on the host.
 - SBUF x layout: strip s holds input rows 32s..32s+31. Partition
   64*img + 32*q + ci where q splits the image WIDTH: q=0 holds cols 0..257,
   q=1 holds cols 256..511 (2-col overlap for the conv halo). Free offset
   264*(r%32) + wc. One [128, 8448] bf16 tile per strip covers both images.
 - Compute: per output row y and width-half q, one 9-MM accumulation chain
   (one [K=32ci, M=32co] matmul per (kh, kw) tap, ~255 output cols each).
   Every chain stays on a single PE tile (tile_position fixed), and every
   PSUM bank is only ever written by one row-quadrant of the PE array --
   both are hard HW requirements (mixing row tiles within an accumulation
   group or a bank crashes the runtime).
 - Blocks of 8 output rows: chain for row y0 + 4h' + c, half q lives in
   PSUM bank (img, q), partitions 32c, cols 256h'. 4 banks per block x2
   double buffer = all 8 banks. 16 chains run concurrently across the 16
   (32x32) PE tiles, all 100%-useful MACs.
 - Drains are full [128, 512] bias-adds psum->bf16, alternating ScalarE
   (img0) / VectorE (img1). Row block 63 is y0=502 (recomputes rows
   502/503) so all drains stay uniform; host takes rows 504..509 from it.
"""
import numpy as np
from ml_dtypes import bfloat16

import concourse.bass as bass
import concourse.tile as tile
from concourse import bacc, mybir
from concourse.bass_utils import run_bass_kernel_spmd
from contextlib import ExitStack

F32 = mybir.dt.float32
BF16 = mybir.dt.bfloat16

N_FULL, C, H, W = 16, 32, 512, 512
HO = WO = 510
N_CORES = 8
N_PER = N_FULL // N_CORES          # 2 images per core
N_STRIPS = H // 32                 # 16 strips of 32 input rows
N_BLOCKS = 64                      # 8-output-row blocks (block 63: y0=502)
RP = 264                           # free row pitch (bf16 elems) per partition
QW = (256, 254)                    # output cols per width-half chain


def _block_y0(m):
    return 8 * m if m < N_BLOCKS - 1 else 502


def _build():
    nc = bacc.Bacc("TRN2", target_bir_lowering=False, debug=False, num_devices=1)
    x_d = nc.dram_tensor("x", [N_STRIPS, 128, 32 * RP], BF16,
                         kind="ExternalInput").ap()
    w_d = nc.dram_tensor("w", [128, 288], BF16, kind="ExternalInput").ap()
    b_d = nc.dram_tensor("b", [128, 1], F32, kind="ExternalInput").ap()
    o_d = nc.dram_tensor("out", [4 * N_BLOCKS, 128, 512], BF16,
                         kind="ExternalOutput").ap()

    with tile.TileContext(nc) as tc, ExitStack() as ctx:
        const_pool = ctx.enter_context(tc.tile_pool(name="const", bufs=1))
        x_pool = ctx.enter_context(tc.tile_pool(name="xs", bufs=3))
        ps_pool = ctx.enter_context(tc.tile_pool(name="ps", bufs=2, space="PSUM"))
        o_pool = ctx.enter_context(tc.tile_pool(name="ob", bufs=3))

        wt = const_pool.tile([128, 288], BF16)
        nc.sync.dma_start(wt[:], w_d[:])
        bt = const_pool.tile([128, 1], F32)
        nc.sync.dma_start(bt[:], b_d[:])

        xtiles = {}

        def load_strip(s):
            xa = x_pool.tile([128, 32 * RP], BF16, tag="x", name=f"xs_{s}")
            nc.scalar.dma_start(xa[:], x_d[s])
            xtiles[s] = xa

        def emit_block(m):
            y0 = _block_y0(m)
            banks = {}
            for il in range(N_PER):
                for q in range(2):
                    banks[(il, q)] = ps_pool.tile(
                        [128, 512], F32, tag=f"ps{il}{q}", name=f"ps{il}{q}_{m}")
            for h in range(2):
                for step in range(9):
                    kh, kw = divmod(step, 3)
                    for il in range(N_PER):
                        for c in range(4):
                            for q in range(2):
                                y = y0 + 4 * h + c
                                r = y + kh
                                st, lr = divmod(r, 32)
                                xa = xtiles[st]
                                pb = 64 * il + 32 * q
                                w_ = QW[q]
                                nc.tensor.matmul(
                                    banks[(il, q)][32 * c:32 * c + 32,
                                                   256 * h:256 * h + w_],
                                    wt[pb:pb + 32,
                                       32 * (3 * kh + kw):32 * (3 * kh + kw) + 32],
                                    xa[pb:pb + 32, RP * lr + kw:RP * lr + kw + w_],
                                    start=(step == 0), stop=(step == 8),
                                    skip_group_check=True,
                                    tile_position=(pb, 32 * c),
                                )
            for il in range(N_PER):
                for q in range(2):
                    ob = o_pool.tile([128, 512], BF16, tag=f"ob{il}{q}",
                                     name=f"ob{il}{q}_{m}")
                    src = banks[(il, q)][0:128, 0:512]
                    if il == 0:
                        nc.scalar.activation(
                            ob[:], src, mybir.ActivationFunctionType.Identity,
                            bias=bt[:])
                    else:
                        nc.vector.tensor_scalar_add(ob[:], src, bt[:])
                    nc.sync.dma_start(o_d[4 * m + 2 * il + q], ob[:])

        load_strip(0)
        for s in range(N_STRIPS):
            if s + 1 < N_STRIPS:
                load_strip(s + 1)
            for m in range(4 * s, min(4 * s + 4, N_BLOCKS)):
                emit_block(m)

    nc.compile()
    return nc


def _prep_inputs(x, weight, bias):
    """Host-side shard + relayout. Returns per-core in_maps."""
    x = np.asarray(x, dtype=np.float32)
    weight = np.asarray(weight, dtype=np.float32)
    bias = np.asarray(bias, dtype=np.float32)

    xb = x.astype(bfloat16)
    # xs[core, s, 64*il+32*q+ci, 264*lr + wc]:
    #   q=0: wc in [0,258) -> x[2*core+il, ci, 32s+lr, wc]
    #   q=1: wc in [0,256) -> x[2*core+il, ci, 32s+lr, 256+wc]
    xs = np.zeros((N_CORES, N_STRIPS, 2, 2, C, 32, RP), dtype=bfloat16)
    xv = xb.reshape(N_CORES, N_PER, C, N_STRIPS, 32, W)
    xv = xv.transpose(0, 3, 1, 2, 4, 5)       # core, s, il, ci, lr, w
    xs[:, :, :, 0, :, :, 0:258] = xv[:, :, :, :, :, 0:258]
    xs[:, :, :, 1, :, :, 0:256] = xv[:, :, :, :, :, 256:512]
    xs = xs.reshape(N_CORES, N_STRIPS, 128, 32 * RP)

    # wt[32g+ci, 32*(3kh+kw)+co] = weight[co, ci, kh, kw], replicated 4x
    wt = weight.transpose(1, 2, 3, 0).reshape(C, 9 * C)
    wt = np.tile(wt, (4, 1)).astype(bfloat16)
    bt = np.tile(bias, 4)[:, None].astype(np.float32)

    return [{"x": xs[i], "w": wt, "b": bt} for i in range(N_CORES)]


def _unpack_output(results):
    """results: list of 8 dicts with 'out' [256, 128, 512] bf16."""
    dev = np.stack([r["out"] for r in results], axis=0)
    # [core, m, il, q, c, co, h, wc]
    dev = dev.reshape(N_CORES, N_BLOCKS, N_PER, 2, 4, C, 2, 256)
    out = np.empty((N_FULL, C, HO, WO), dtype=np.float32)
    o = out.reshape(N_CORES, N_PER, C, HO, WO)
    # row claimed by (m, h, c) = 8m + 4h + c (true for m < 63)
    for q, (w0, w1) in ((0, (0, 256)), (1, (256, 510))):
        dq = dev[:, :, :, q]                     # core, m, il, c, co, h, wc
        full = dq.transpose(0, 2, 4, 1, 5, 3, 6) # core, il, co, m, h, c, wc
        full = full.reshape(N_CORES, N_PER, C, 512, 256)
        o[:, :, :, :504, w0:w1] = full[:, :, :, :504, :w1 - w0]
        # block 63 holds rows 502 + 4h + c; rows 504..509 are entries 2..7
        lq = dq[:, 63].transpose(0, 1, 3, 4, 2, 5)   # core, il, co, h, c, wc
        lq = lq.reshape(N_CORES, N_PER, C, 8, 256)
        o[:, :, :, 504:510, w0:w1] = lq[:, :, :, 2:8, :w1 - w0]
    return out


_NC = None


def kernel(x, weight, bias):
    global _NC
    if _NC is None:
        _NC = _build()
    in_maps = _prep_inputs(x, weight, bias)
    res = run_bass_kernel_spmd(_NC, in_maps, core_ids=list(range(N_CORES)))
    return _unpack_output(res.results)


# revision 5
# speedup vs baseline: 2.0720x; 2.0720x over previous
"""Trainium2 Bass kernel for CustomConv2d:
  x [16, 32, 512, 512] f32, weight [32, 32, 3, 3] f32, bias [32] f32
  -> out [16, 32, 510, 510] f32   (stride 1, VALID padding, + bias)

Data-parallel over batch: 2 images per core across 8 NeuronCores.

v5 design (bf16 I/O + 4-tile [64,64] PE, tile-pure chains):
 - Host converts x/weight to bf16 and relayouts x into the exact SBUF strip
   layout, so every input DMA is one fully-contiguous 2MB transfer per strip
   tile. Output is dumped as packed [128, 510] bf16 drain tiles and
   unshuffled + upcast to f32 on the host.
 - SBUF x layout: strip s holds input rows 32s..32s+31; partition
   64*img + 32*(r%2) + ci, free offset 512*((r%32)//2) + w. One [128, 8192]
   bf16 tile per strip covers both images of the core.
 - Compute: per output row pair (y, y+1), y even: one 6-MM accumulation
   chain of [K=64, M=64] matmuls -- 3 kw taps x 2 two-row input windows,
   with 3-of-4 weight blocks useful per MM (75%, the ceiling for 2-row
   windows). Chains are tile-pure and banks row-half-pure: HW crashes if an
   accumulation group spans PE row tiles or a PSUM bank is written by more
   than one row quadrant (verified by probing), and the PE sustains only ~8
   concurrent matmul streams, so 4 tiles of [64,64] keep the whole array
   busy with big, cheap-to-issue matmuls (3072 total; LDWEIGHTS has ~90ns
   fixed cost and ~2.6x concurrency, so 18k+ small matmuls are issue-bound).
 - Blocks of 4 output rows (2 pairs): pair cp -> PSUM bank (il) partitions
   64cp+32h+co = row y0+2cp+h. 2 banks per block, bufs=3 -> 6 of 8 banks.
 - Drains are full [128, 510] bias-adds psum->bf16, alternating ScalarE
   (img0) / VectorE (img1). Row block 127 is y0=506 (recomputes rows
   506/507) so all drains stay uniform; host takes rows 508/509 from it.
"""
import numpy as np
from ml_dtypes import bfloat16

import concourse.bass as bass
import concourse.tile as tile
from concourse import bacc, mybir
from concourse.bass_utils import run_bass_kernel_spmd
from contextlib import ExitStack

F32 = mybir.dt.float32
BF16 = mybir.dt.bfloat16

N_FULL, C, H, W = 16, 32, 512, 512
HO = WO = 510
N_CORES = 8
N_PER = N_FULL // N_CORES          # 2 images per core
N_STRIPS = H // 32                 # 16 strips of 32 input rows
N_BLOCKS = 128                     # 4-output-row blocks (block 127: y0=506)


def _block_y0(mb):
    return 4 * mb if mb < N_BLOCKS - 1 else 506


def _build():
    nc = bacc.Bacc("TRN2", target_bir_lowering=False, debug=False, num_devices=1)
    x_d = nc.dram_tensor("x", [N_STRIPS, 128, 8192], BF16,
                         kind="ExternalInput").ap()
    w_d = nc.dram_tensor("w", [128, 384], BF16, kind="ExternalInput").ap()
    b_d = nc.dram_tensor("b", [128, 1], F32, kind="ExternalInput").ap()
    o_d = nc.dram_tensor("out", [2 * N_BLOCKS, 128, WO], BF16,
                         kind="ExternalOutput").ap()

    with tile.TileContext(nc) as tc, ExitStack() as ctx:
        const_pool = ctx.enter_context(tc.tile_pool(name="const", bufs=1))
        x_pool = ctx.enter_context(tc.tile_pool(name="xs", bufs=3))
        ps_pool = ctx.enter_context(tc.tile_pool(name="ps", bufs=3, space="PSUM"))
        o_pool = ctx.enter_context(tc.tile_pool(name="ob", bufs=3))

        wv = const_pool.tile([128, 384], BF16)
        nc.sync.dma_start(wv[:], w_d[:])
        bt = const_pool.tile([128, 1], F32)
        nc.sync.dma_start(bt[:], b_d[:])

        xtiles = {}

        def load_strip(s):
            xa = x_pool.tile([128, 8192], BF16, tag="x", name=f"xs_{s}")
            nc.scalar.dma_start(xa[:], x_d[s])
            xtiles[s] = xa

        def emit_block(mb):
            y0 = _block_y0(mb)
            banks = {}
            for il in range(N_PER):
                banks[il] = ps_pool.tile([128, 512], F32, tag=f"ps{il}",
                                         name=f"ps{il}_{mb}")
            for step in range(6):
                w, kw = divmod(step, 3)
                for il in range(N_PER):
                    for cp in range(2):
                        rw = y0 + 2 * cp + 2 * w       # window rows rw, rw+1
                        st, lrw = divmod(rw, 32)
                        t = lrw // 2
                        xa = xtiles[st]
                        nc.tensor.matmul(
                            banks[il][64 * cp:64 * cp + 64, 0:WO],
                            wv[64 * il:64 * il + 64,
                               64 * (3 * w + kw):64 * (3 * w + kw) + 64],
                            xa[64 * il:64 * il + 64, 512 * t + kw:512 * t + kw + WO],
                            start=(step == 0), stop=(step == 5),
                            skip_group_check=True,
                            tile_position=(64 * il, 64 * cp),
                        )
            for il in range(N_PER):
                ob = o_pool.tile([128, WO], BF16, tag=f"ob{il}",
                                 name=f"ob{il}_{mb}")
                src = banks[il][0:128, 0:WO]
                if il == 0:
                    nc.scalar.activation(
                        ob[:], src, mybir.ActivationFunctionType.Identity,
                        bias=bt[:])
                else:
                    nc.vector.tensor_scalar_add(ob[:], src, bt[:])
                nc.sync.dma_start(o_d[2 * mb + il], ob[:])

        load_strip(0)
        for s in range(N_STRIPS):
            if s + 1 < N_STRIPS:
                load_strip(s + 1)
            for mb in range(8 * s, min(8 * s + 8, N_BLOCKS)):
                emit_block(mb)

    nc.compile()
    return nc


def _prep_inputs(x, weight, bias):
    """Host-side shard + relayout. Returns per-core in_maps."""
    x = np.asarray(x, dtype=np.float32)
    weight = np.asarray(weight, dtype=np.float32)
    bias = np.asarray(bias, dtype=np.float32)

    # x[2i+il, ci, 32s+2t+q, w] -> xs[i, s, 64*il+32*q+ci, 512*t+w]
    xr = x.reshape(N_CORES, N_PER, C, N_STRIPS, 16, 2, W)
    xr = xr.transpose(0, 3, 1, 5, 2, 4, 6)          # core, s, il, q, ci, t, w
    xs = np.ascontiguousarray(xr).reshape(N_CORES, N_STRIPS, 128, 8192)
    xs = xs.astype(bfloat16)

    # wv[64il + 32q + ci, 64*(3w+kw) + 32h + co] = weight[co, ci, 2w+q-h, kw]
    # (zero when kh = 2w+q-h is outside [0, 3))
    wk = np.zeros((2, 32, 6, 2, 32), dtype=np.float32)  # q, ci, (w,kw), h, co
    for w in range(2):
        for kw in range(3):
            for q in range(2):
                for h in range(2):
                    kh = 2 * w + q - h
                    if 0 <= kh <= 2:
                        wk[q, :, 3 * w + kw, h, :] = weight[:, :, kh, kw].T
    wv = wk.transpose(0, 1, 2, 3, 4).reshape(64, 384)
    wv = np.tile(wv, (2, 1)).astype(bfloat16)
    bt = np.tile(bias, 4)[:, None].astype(np.float32)

    return [{"x": xs[i], "w": wv, "b": bt} for i in range(N_CORES)]


def _unpack_output(results):
    """results: list of 8 dicts with 'out' [256, 128, 510] bf16."""
    dev = np.stack([r["out"] for r in results], axis=0)
    # [core, mb, il, cp, h, co, w]
    dev = dev.reshape(N_CORES, N_BLOCKS, N_PER, 2, 2, C, WO)
    out = np.empty((N_FULL, C, HO, WO), dtype=np.float32)
    o = out.reshape(N_CORES, N_PER, C, HO, WO)
    # row claimed by (mb, cp, h) = 4mb + 2cp + h (true for mb < 127)
    full = dev.transpose(0, 2, 5, 1, 3, 4, 6).reshape(N_CORES, N_PER, C, 512, WO)
    o[:, :, :, :508, :] = full[:, :, :, :508, :]
    # block 127 holds rows 506 + 2cp + h; rows 508/509 are entries 2/3
    lb = dev[:, 127].transpose(0, 1, 4, 2, 3, 5).reshape(N_CORES, N_PER, C, 4, WO)
    o[:, :, :, 508:510, :] = lb[:, :, :, 2:4, :]
    return out


_NC = None


def kernel(x, weight, bias):
    global _NC
    if _NC is None:
        _NC = _build()
    in_maps = _prep_inputs(x, weight, bias)
    res = run_bass_kernel_spmd(_NC, in_maps, core_ids=list(range(N_CORES)))
    return _unpack_output(res.results)
